# revision 53
# baseline (speedup 1.0000x reference)
"""CConv (continuous conv / GNN message passing) Trainium2 Bass kernel.

Math (per point n):
    pf[n,m,:]  = feat_in[neighbor_idx[n,m], :]                 # gather
    t[n,s,i]   = sum_m select_mat[n,m,s] * pf[n,m,i]           # stage 1
    out[n,o]   = sum_{s,i} t[n,s,i] * W[s,o,i]                 # stage 2

Strategy: data-parallel over points across 8 cores; per core, 49 groups of
128 points (32 blocks of 4 points). The neighbor gather runs host-side and
ships as an fp8e3 (e3m4) stream — half the HBM traffic of bf16; the fp8
scale folds into the stage-2 weight. pf and select arrive in 4-group DMA
chunks (~2 MB) so the SDMA engines run near line rate, issued only from
near-idle queues (sync/gpsimd) because an HWDGE issue parked in a busy
engine FIFO delays the prefetch by a whole batch. Two select paths:
SEL_FP8BD ships the select pre-block-diagonalized as fp8e3 (zero on-chip
prep); otherwise it ships dense bf16 and per-batch 4x-accelerated DVE
scatter copies build one of four persistent pre-zeroed block-diag
operands. Stage 1 is one matmul per 4-point block (lhsT = pf fp8, rhs =
112 block-diag cols) streaming at the PE's back-to-back rate into 2-bank
PSUM tiles (block stride padded to 128 cols so MM writes stay contiguous);
one 3-dim copy per tile drains both banks into a 4-group staging tile.
Stage 2 batches 4 groups: per spatial tap one N=512 matmul with the weight
stationary, accumulating in a full PSUM bank; the [O, points] output
layout is transposed back on the host.
"""
import sys

sys.path.insert(0, '/opt/trn_rl_repo')

import numpy as np
import ml_dtypes

import concourse.bass as bass
import concourse.tile as tile
from concourse import bacc, mybir
from concourse.bass_utils import run_bass_kernel_spmd

BF16 = ml_dtypes.bfloat16
F8E3 = ml_dtypes.float8_e3m4

N = 50000
M = 32            # neighbors per point
S = 27            # spatial bins
SP = 28           # padded spatial
I = 128           # in channels
O = 128           # out channels
NCORES = 8
NPAD = 50176      # 8 * 49 * 128
NPC = NPAD // NCORES        # 6272 points per core
G = NPC // 128              # 49 groups of 128 points
B = 32                      # 4-point blocks per group
SUB = 4                     # blocks per PSUM bank
BD = 4 * SP                 # block-diag columns per block (112)
GB = 4                      # groups per stage-2 batch / DMA chunk
NB = (G + GB - 1) // GB     # 13 batches (last has 1 group)
GPAD = NB * GB              # 52 groups incl. padding for whole DMA chunks

SEL_FP8BD = True            # ship select pre-block-diagonalized in fp8e3
SEL_BETA = 15.5             # select fp8 scale (only for SEL_FP8BD)


def build_nc(sel_fp8bd):
    nc = bacc.Bacc("TRN2", target_bir_lowering=False, debug=False)

    pfp = nc.dram_tensor("pfp", [NB, 128, GB * B * I], mybir.dt.uint8,
                         kind="ExternalInput")
    if sel_fp8bd:
        selp = nc.dram_tensor("selp", [NB, 128, GB * B * BD], mybir.dt.uint8,
                              kind="ExternalInput")
        SELW = B * BD
    else:
        selp = nc.dram_tensor("selp", [NB, 128, GB * B * SP], mybir.dt.bfloat16,
                              kind="ExternalInput")
        SELW = B * SP
    wt = nc.dram_tensor("wt", [I, S * O], mybir.dt.bfloat16, kind="ExternalInput")
    zs = nc.dram_tensor("zs", [128, B * BD], mybir.dt.bfloat16, kind="ExternalInput")
    outp = nc.dram_tensor("outp", [O, NPC], mybir.dt.bfloat16, kind="ExternalOutput")

    with tile.TileContext(nc) as tc:
        with (
            tc.tile_pool(name="const", bufs=1) as const_pool,
            tc.tile_pool(name="pfbuf", bufs=3) as pfbuf,
            tc.tile_pool(name="selbuf", bufs=3) as selbuf,
            tc.tile_pool(name="tgbuf", bufs=2) as tgbuf,
            tc.tile_pool(name="otbuf", bufs=2) as otbuf,
            tc.tile_pool(name="psum1", bufs=5, space="PSUM") as psum1,
            tc.tile_pool(name="psum2", bufs=3, space="PSUM") as psum2,
        ):
            wt_t = const_pool.tile([128, S * O], mybir.dt.bfloat16)
            nc.scalar.dma_start(out=wt_t[:], in_=wt[:])
            bd = None
            if not sel_fp8bd:
                # four persistent block-diag select operands (one per group
                # of a batch); zeros off the diagonal persist across groups
                bd0 = const_pool.tile([128, B, BD], mybir.dt.bfloat16)
                bd1 = const_pool.tile([128, B, BD], mybir.dt.bfloat16)
                bd2 = const_pool.tile([128, B, BD], mybir.dt.bfloat16)
                bd3 = const_pool.tile([128, B, BD], mybir.dt.bfloat16)
                bd = [bd0, bd1, bd2, bd3]
                nc.vector.memset(bd0[:], 0.0)
                nc.gpsimd.memset(bd1[:], 0.0)
                nc.scalar.copy(out=bd2[:], in_=bd0[:])
                nc.scalar.dma_start(out=bd3[:], in_=zs[:])

            def stage2(k, Tg4, ngrp):
                po = psum2.tile([128, GB * 128], mybir.dt.float32, space="PSUM")
                ncols = ngrp * 128
                for s in range(S):
                    nc.tensor.matmul(
                        out=po[:, 0:ncols],
                        lhsT=wt_t[:, s * O:(s + 1) * O],
                        rhs=Tg4[:, s, 0:ncols],
                        start=(s == 0), stop=(s == S - 1),
                    )
                ot = otbuf.tile([128, GB * 128], mybir.dt.bfloat16)
                if k % 2 == 0:
                    nc.vector.tensor_copy(out=ot[:, 0:ncols], in_=po[:, 0:ncols])
                else:
                    nc.scalar.copy(out=ot[:, 0:ncols], in_=po[:, 0:ncols])
                nc.sync.dma_start(out=outp[:, k * GB * 128:k * GB * 128 + ncols],
                                  in_=ot[:, 0:ncols])

            def alloc_tiles():
                pf_t = pfbuf.tile([128, GB, B, I], mybir.dt.uint8)
                if sel_fp8bd:
                    sel_t = selbuf.tile([128, GB, B, BD], mybir.dt.uint8)
                else:
                    sel_t = selbuf.tile([128, GB, B, SP], mybir.dt.bfloat16)
                return pf_t, sel_t

            # batch 0: per-group slices on the two HWDGE rings so group 0's
            # operands land in a few us (nothing else contends yet)
            cur = alloc_tiles()
            for gq in range(GB):
                nc.scalar.dma_start(
                    out=cur[1][:, gq, :, :],
                    in_=selp[0][:, gq * SELW:(gq + 1) * SELW])
                nc.sync.dma_start(
                    out=cur[0][:, gq, :, :],
                    in_=pfp[0][:, gq * B * I:(gq + 1) * B * I])

            prev = None  # (batch index, Tg4 tile, ngroups)
            for k in range(NB):
                g0 = k * GB
                ngrp = min(GB, G - g0)
                pf_t, sel_t = cur

                Tg4 = tgbuf.tile([128, S, GB * 128], mybir.dt.bfloat16)
                if not sel_fp8bd:
                    # hoisted scatter burst: bd[gq][q, b, (q//32)*28 + s] =
                    # sel[q, gq, b, s]; keeps the DVE FIFO free of
                    # head-of-line waits on this batch's matmuls
                    for gq in range(ngrp):
                        for nb in range(4):
                            src = sel_t[32 * nb:32 * (nb + 1), gq, :, :]
                            dst = bd[gq][32 * nb:32 * (nb + 1), :,
                                         nb * SP:(nb + 1) * SP]
                            nc.vector.tensor_copy(out=dst, in_=src)
                for gq in range(ngrp):
                    g = g0 + gq
                    for c in range(B // SUB):
                        # 1-bank tile; block sub lives at col sub*128 (112
                        # data + 16 pad) -> contiguous MM writes AND a
                        # uniform-stride drain; 6 rotating banks + short
                        # per-bank drains keep the PE from stalling on
                        # PSUM recycling
                        pt = psum1.tile([128, 4, 128], mybir.dt.float32,
                                        space="PSUM")
                        for sub in range(SUB):
                            b = c * SUB + sub
                            if sel_fp8bd:
                                rhs_ap = sel_t[:, gq, b, :].bitcast(mybir.dt.float8e3)
                            else:
                                rhs_ap = bd[gq][:, b, :]
                            nc.tensor.matmul(
                                out=pt[:, sub, 0:BD],
                                lhsT=pf_t[:, gq, b, :].bitcast(mybir.dt.float8e3),
                                rhs=rhs_ap,
                                start=True, stop=True,
                            )
                        # drain: src col = sub*128 + nb*28 + s,
                        # dst col = s*512 + gq*128 + (c*4+sub)*4 + nb
                        src_ap = bass.AP(tensor=pt.tensor, offset=pt[:].offset,
                                         ap=[pt[:].ap[0], [1, S], [128, 4], [SP, 4]])
                        dst_ap = bass.AP(tensor=Tg4.tensor,
                                         offset=Tg4[:].offset + gq * 128 + c * 16,
                                         ap=[Tg4[:].ap[0], [GB * 128, S], [4, 4], [1, 4]])
                        if sel_fp8bd:
                            on_dve = c % 2 == 0
                        else:
                            # DVE runs the scatters too -> 3 of 8 drains
                            on_dve = c in (1, 4, 6)
                        if on_dve:
                            nc.vector.tensor_copy(out=dst_ap, in_=src_ap)
                        else:
                            nc.scalar.copy(out=dst_ap, in_=src_ap)

                    if gq == (0 if k == 0 else min(1, ngrp - 1)):
                        if k + 1 < NB:
                            # prefetch the next chunk now. k==0 issues from
                            # the ACT queue: its FIFO delays the transfer
                            # behind batch 0's first drains, so the big
                            # chunk doesn't steal SDMA bandwidth from batch
                            # 0's startup slices. Later batches use SWDGE
                            # (latency prefetch-hidden; pool WAR gates it).
                            nxt = alloc_tiles()
                            ring = nc.scalar if k == 0 else nc.gpsimd
                            ring.dma_start(out=nxt[0][:], in_=pfp[k + 1])
                            ring.dma_start(out=nxt[1][:], in_=selp[k + 1])
                            cur = nxt
                        if prev is not None:
                            stage2(*prev)
                prev = (k, Tg4, ngrp)
            stage2(*prev)

    nc.compile()
    return nc


_NC = {}


def get_nc(sel_fp8bd):
    if sel_fp8bd not in _NC:
        _NC[sel_fp8bd] = build_nc(sel_fp8bd)
    return _NC[sel_fp8bd]


def make_in_maps(feat_in, select_mat, weight, neighbor_idx, sel_fp8bd):
    feat = np.asarray(feat_in, dtype=np.float32)
    alpha = 15.5 / max(float(np.abs(feat).max()), 1e-30)
    feat8u = (feat * alpha).astype(F8E3).view(np.uint8)

    sel = np.asarray(select_mat, dtype=np.float32)
    sel_src = np.zeros((NPAD, M, S), dtype=np.float32)
    sel_src[:N] = sel

    nidx = np.asarray(neighbor_idx).astype(np.int64)
    idx_pad = np.zeros((NPAD, M), dtype=np.int64)
    idx_pad[:N] = nidx

    beta = SEL_BETA if sel_fp8bd else 1.0
    w = np.asarray(weight, dtype=np.float32)
    # wt[i, s*O+o] = W[s, o, i] / (alpha * beta)
    wt_np = np.ascontiguousarray(
        (w / (alpha * beta)).reshape(S, O, I).transpose(2, 0, 1).reshape(I, S * O)
    ).astype(BF16)

    in_maps = []
    for core in range(NCORES):
        selc = np.zeros((GPAD * 128, M, S), dtype=np.float32)
        selc[:NPC] = sel_src[core * NPC:(core + 1) * NPC]
        if sel_fp8bd:
            sel8 = (selc * beta).astype(F8E3).view(np.uint8)
            # selp[k, nb*32+m, ((gq*B)+b)*BD + nb*SP + s] =
            #     sel8[(4k+gq)*128 + b*4 + nb, m, s]
            sc = sel8.reshape(NB, GB, B, 4, M, S)
            bdarr = np.zeros((NB, 4, M, GB, B, 4, SP), dtype=np.uint8)
            for nb in range(4):
                bdarr[:, nb, :, :, :, nb, :S] = sc[:, :, :, nb, :, :].transpose(
                    0, 3, 1, 2, 4)
            selp_np = bdarr.reshape(NB, 128, GB * B * BD)
        else:
            selp_pad = np.zeros((GPAD * 128, M, SP), dtype=BF16)
            selp_pad[:, :, :S] = selc.astype(BF16)
            # selp[k, nb*32+m, ((gq*B)+b)*SP + s]
            sc = selp_pad.reshape(NB, GB, B, 4, M, SP)
            selp_np = np.ascontiguousarray(
                sc.transpose(0, 3, 4, 1, 2, 5)).reshape(NB, 128, GB * B * SP)

        idxc = np.zeros((GPAD * 128, M), dtype=np.int64)
        idxc[:NPC] = idx_pad[core * NPC:(core + 1) * NPC]
        # idxp[k, nb*32+m, gq*B+b] = neighbor_idx[(4k+gq)*128 + b*4 + nb, m]
        idxp = np.ascontiguousarray(
            idxc.reshape(NB, GB, B, 4, M).transpose(0, 3, 4, 1, 2)
        ).reshape(NB, 128, GB * B)
        pfp_np = feat8u[idxp].reshape(NB, 128, GB * B * I)
        in_maps.append({
            "pfp": pfp_np,
            "selp": selp_np,
            "wt": wt_np,
            "zs": np.zeros((128, B * BD), dtype=BF16),
        })
    return in_maps


def run(feat_in, select_mat, weight, neighbor_idx, trace=False):
    nc = get_nc(SEL_FP8BD)
    in_maps = make_in_maps(feat_in, select_mat, weight, neighbor_idx, SEL_FP8BD)
    res = run_bass_kernel_spmd(nc, in_maps, core_ids=list(range(NCORES)), trace=trace)
    outs = [res.results[c]["outp"] for c in range(NCORES)]   # each [O, NPC]
    full = np.concatenate(outs, axis=1).astype(np.float32).T[:N]   # [N, O]
    return np.ascontiguousarray(full)[:, :, None], res


def kernel(feat_in, select_mat, weight, neighbor_idx):
    out, _ = run(feat_in, select_mat, weight, neighbor_idx, trace=False)
    return out


# revision 55
# speedup vs baseline: 1.0181x; 1.0181x over previous
"""CConv (continuous conv / GNN message passing) Trainium2 Bass kernel.

Math (per point n):
    pf[n,m,:]  = feat_in[neighbor_idx[n,m], :]                 # gather
    t[n,s,i]   = sum_m select_mat[n,m,s] * pf[n,m,i]           # stage 1
    out[n,o]   = sum_{s,i} t[n,s,i] * W[s,o,i]                 # stage 2

Strategy: data-parallel over points across 8 cores; per core, 49 groups of
128 points (32 blocks of 4 points). The neighbor gather runs host-side and
ships as an fp8e3 (e3m4) stream — half the HBM traffic of bf16; the fp8
scale folds into the stage-2 weight. pf and select arrive in 4-group DMA
chunks (~2 MB) so the SDMA engines run near line rate, issued only from
near-idle queues (sync/gpsimd) because an HWDGE issue parked in a busy
engine FIFO delays the prefetch by a whole batch. Two select paths:
SEL_FP8BD ships the select pre-block-diagonalized as fp8e3 (zero on-chip
prep); otherwise it ships dense bf16 and per-batch 4x-accelerated DVE
scatter copies build one of four persistent pre-zeroed block-diag
operands. Stage 1 is one matmul per 4-point block (lhsT = pf fp8, rhs =
112 block-diag cols) streaming at the PE's back-to-back rate into 2-bank
PSUM tiles (block stride padded to 128 cols so MM writes stay contiguous);
one 3-dim copy per tile drains both banks into a 4-group staging tile.
Stage 2 batches 4 groups: per spatial tap one N=512 matmul with the weight
stationary, accumulating in a full PSUM bank; the [O, points] output
layout is transposed back on the host.
"""
import sys

sys.path.insert(0, '/opt/trn_rl_repo')

import numpy as np
import ml_dtypes

import concourse.bass as bass
import concourse.tile as tile
from concourse import bacc, mybir
from concourse.bass_utils import run_bass_kernel_spmd

BF16 = ml_dtypes.bfloat16
F8E3 = ml_dtypes.float8_e3m4

N = 50000
M = 32            # neighbors per point
S = 27            # spatial bins
SP = 28           # padded spatial
I = 128           # in channels
O = 128           # out channels
NCORES = 8
NPAD = 50176      # 8 * 49 * 128
NPC = NPAD // NCORES        # 6272 points per core
G = NPC // 128              # 49 groups of 128 points
B = 32                      # 4-point blocks per group
SUB = 4                     # blocks per PSUM bank
BD = 4 * SP                 # block-diag columns per block (112)
GB = 4                      # groups per stage-2 batch / DMA chunk
NB = (G + GB - 1) // GB     # 13 batches (last has 1 group)
GPAD = NB * GB              # 52 groups incl. padding for whole DMA chunks

SEL_FP8BD = True            # ship select pre-block-diagonalized in fp8e3
SEL_BETA = 15.5             # select fp8 scale (only for SEL_FP8BD)


def build_nc(sel_fp8bd):
    nc = bacc.Bacc("TRN2", target_bir_lowering=False, debug=False)

    pfp = nc.dram_tensor("pfp", [NB, 128, GB * B * I], mybir.dt.uint8,
                         kind="ExternalInput")
    if sel_fp8bd:
        selp = nc.dram_tensor("selp", [NB, 128, GB * B * BD], mybir.dt.uint8,
                              kind="ExternalInput")
        SELW = B * BD
    else:
        selp = nc.dram_tensor("selp", [NB, 128, GB * B * SP], mybir.dt.bfloat16,
                              kind="ExternalInput")
        SELW = B * SP
    wt = nc.dram_tensor("wt", [I, S * O], mybir.dt.bfloat16, kind="ExternalInput")
    zs = nc.dram_tensor("zs", [128, B * BD], mybir.dt.bfloat16, kind="ExternalInput")
    outp = nc.dram_tensor("outp", [O, NPC], mybir.dt.bfloat16, kind="ExternalOutput")

    with tile.TileContext(nc) as tc:
        with (
            tc.tile_pool(name="const", bufs=1) as const_pool,
            tc.tile_pool(name="pfbuf", bufs=2) as pfbuf,
            tc.tile_pool(name="selbuf", bufs=2) as selbuf,
            tc.tile_pool(name="tgbuf", bufs=2) as tgbuf,
            tc.tile_pool(name="otbuf", bufs=2) as otbuf,
            tc.tile_pool(name="psum1", bufs=5, space="PSUM") as psum1,
            tc.tile_pool(name="psum2", bufs=3, space="PSUM") as psum2,
        ):
            wt_t = const_pool.tile([128, S * O], mybir.dt.bfloat16)
            nc.scalar.dma_start(out=wt_t[:], in_=wt[:])
            bd = None
            if not sel_fp8bd:
                # four persistent block-diag select operands (one per group
                # of a batch); zeros off the diagonal persist across groups
                bd0 = const_pool.tile([128, B, BD], mybir.dt.bfloat16)
                bd1 = const_pool.tile([128, B, BD], mybir.dt.bfloat16)
                bd2 = const_pool.tile([128, B, BD], mybir.dt.bfloat16)
                bd3 = const_pool.tile([128, B, BD], mybir.dt.bfloat16)
                bd = [bd0, bd1, bd2, bd3]
                nc.vector.memset(bd0[:], 0.0)
                nc.gpsimd.memset(bd1[:], 0.0)
                nc.scalar.copy(out=bd2[:], in_=bd0[:])
                nc.scalar.dma_start(out=bd3[:], in_=zs[:])

            def stage2(k, Tg4, ngrp):
                po = psum2.tile([128, GB * 128], mybir.dt.float32, space="PSUM")
                ncols = ngrp * 128
                for s in range(S):
                    nc.tensor.matmul(
                        out=po[:, 0:ncols],
                        lhsT=wt_t[:, s * O:(s + 1) * O],
                        rhs=Tg4[:, s, 0:ncols],
                        start=(s == 0), stop=(s == S - 1),
                    )
                ot = otbuf.tile([128, GB * 128], mybir.dt.bfloat16)
                if k % 2 == 0:
                    nc.vector.tensor_copy(out=ot[:, 0:ncols], in_=po[:, 0:ncols])
                else:
                    nc.scalar.copy(out=ot[:, 0:ncols], in_=po[:, 0:ncols])
                nc.sync.dma_start(out=outp[:, k * GB * 128:k * GB * 128 + ncols],
                                  in_=ot[:, 0:ncols])

            def alloc_tiles():
                pf_t = pfbuf.tile([128, GB, B, I], mybir.dt.uint8)
                if sel_fp8bd:
                    sel_t = selbuf.tile([128, GB, B, BD], mybir.dt.uint8)
                else:
                    sel_t = selbuf.tile([128, GB, B, SP], mybir.dt.bfloat16)
                return pf_t, sel_t

            # batch 0: per-group slices on the two HWDGE rings so group 0's
            # operands land in a few us (nothing else contends yet)
            cur = alloc_tiles()
            for gq in range(GB):
                nparts = 4 if gq == 0 else 1   # group 0: 8-block sub-slices
                for p in range(nparts):
                    bl, bh = p * B // nparts, (p + 1) * B // nparts
                    nc.scalar.dma_start(
                        out=cur[1][:, gq, bl:bh, :],
                        in_=selp[0][:, gq * SELW + bl * (SELW // B):
                                    gq * SELW + bh * (SELW // B)])
                    nc.sync.dma_start(
                        out=cur[0][:, gq, bl:bh, :],
                        in_=pfp[0][:, (gq * B + bl) * I:(gq * B + bh) * I])

            prev = None  # (batch index, Tg4 tile, ngroups)
            for k in range(NB):
                g0 = k * GB
                ngrp = min(GB, G - g0)
                pf_t, sel_t = cur

                Tg4 = tgbuf.tile([128, S, GB * 128], mybir.dt.bfloat16)
                if not sel_fp8bd:
                    # hoisted scatter burst: bd[gq][q, b, (q//32)*28 + s] =
                    # sel[q, gq, b, s]; keeps the DVE FIFO free of
                    # head-of-line waits on this batch's matmuls
                    for gq in range(ngrp):
                        for nb in range(4):
                            src = sel_t[32 * nb:32 * (nb + 1), gq, :, :]
                            dst = bd[gq][32 * nb:32 * (nb + 1), :,
                                         nb * SP:(nb + 1) * SP]
                            nc.vector.tensor_copy(out=dst, in_=src)
                for gq in range(ngrp):
                    g = g0 + gq
                    for c in range(B // SUB):
                        # 1-bank tile; block sub lives at col sub*128 (112
                        # data + 16 pad) -> contiguous MM writes AND a
                        # uniform-stride drain; 6 rotating banks + short
                        # per-bank drains keep the PE from stalling on
                        # PSUM recycling
                        pt = psum1.tile([128, 4, 128], mybir.dt.float32,
                                        space="PSUM")
                        for sub in range(SUB):
                            b = c * SUB + sub
                            if sel_fp8bd:
                                rhs_ap = sel_t[:, gq, b, :].bitcast(mybir.dt.float8e3)
                            else:
                                rhs_ap = bd[gq][:, b, :]
                            nc.tensor.matmul(
                                out=pt[:, sub, 0:BD],
                                lhsT=pf_t[:, gq, b, :].bitcast(mybir.dt.float8e3),
                                rhs=rhs_ap,
                                start=True, stop=True,
                            )
                        # drain: src col = sub*128 + nb*28 + s,
                        # dst col = s*512 + gq*128 + (c*4+sub)*4 + nb
                        src_ap = bass.AP(tensor=pt.tensor, offset=pt[:].offset,
                                         ap=[pt[:].ap[0], [1, S], [128, 4], [SP, 4]])
                        dst_ap = bass.AP(tensor=Tg4.tensor,
                                         offset=Tg4[:].offset + gq * 128 + c * 16,
                                         ap=[Tg4[:].ap[0], [GB * 128, S], [4, 4], [1, 4]])
                        if sel_fp8bd:
                            on_dve = c % 2 == 0
                        else:
                            # DVE runs the scatters too -> 3 of 8 drains
                            on_dve = c in (1, 4, 6)
                        if on_dve:
                            nc.vector.tensor_copy(out=dst_ap, in_=src_ap)
                        else:
                            nc.scalar.copy(out=dst_ap, in_=src_ap)

                    if gq == (0 if k == 0 else min(1, ngrp - 1)):
                        if k + 1 < NB:
                            # prefetch the next chunk now. k==0 issues from
                            # the ACT queue: its FIFO delays the transfer
                            # behind batch 0's first drains, so the big
                            # chunk doesn't steal SDMA bandwidth from batch
                            # 0's startup slices. Later batches use SWDGE
                            # (latency prefetch-hidden; pool WAR gates it).
                            nxt = alloc_tiles()
                            ring = nc.scalar if k == 0 else nc.gpsimd
                            ring.dma_start(out=nxt[0][:], in_=pfp[k + 1])
                            ring.dma_start(out=nxt[1][:], in_=selp[k + 1])
                            cur = nxt
                        if prev is not None:
                            stage2(*prev)
                prev = (k, Tg4, ngrp)
            stage2(*prev)

    nc.compile()
    return nc


_NC = {}


def get_nc(sel_fp8bd):
    if sel_fp8bd not in _NC:
        _NC[sel_fp8bd] = build_nc(sel_fp8bd)
    return _NC[sel_fp8bd]


def make_in_maps(feat_in, select_mat, weight, neighbor_idx, sel_fp8bd):
    feat = np.asarray(feat_in, dtype=np.float32)
    alpha = 15.5 / max(float(np.abs(feat).max()), 1e-30)
    feat8u = (feat * alpha).astype(F8E3).view(np.uint8)

    sel = np.asarray(select_mat, dtype=np.float32)
    sel_src = np.zeros((NPAD, M, S), dtype=np.float32)
    sel_src[:N] = sel

    nidx = np.asarray(neighbor_idx).astype(np.int64)
    idx_pad = np.zeros((NPAD, M), dtype=np.int64)
    idx_pad[:N] = nidx

    beta = SEL_BETA if sel_fp8bd else 1.0
    w = np.asarray(weight, dtype=np.float32)
    # wt[i, s*O+o] = W[s, o, i] / (alpha * beta)
    wt_np = np.ascontiguousarray(
        (w / (alpha * beta)).reshape(S, O, I).transpose(2, 0, 1).reshape(I, S * O)
    ).astype(BF16)

    in_maps = []
    for core in range(NCORES):
        selc = np.zeros((GPAD * 128, M, S), dtype=np.float32)
        selc[:NPC] = sel_src[core * NPC:(core + 1) * NPC]
        if sel_fp8bd:
            sel8 = (selc * beta).astype(F8E3).view(np.uint8)
            # selp[k, nb*32+m, ((gq*B)+b)*BD + nb*SP + s] =
            #     sel8[(4k+gq)*128 + b*4 + nb, m, s]
            sc = sel8.reshape(NB, GB, B, 4, M, S)
            bdarr = np.zeros((NB, 4, M, GB, B, 4, SP), dtype=np.uint8)
            for nb in range(4):
                bdarr[:, nb, :, :, :, nb, :S] = sc[:, :, :, nb, :, :].transpose(
                    0, 3, 1, 2, 4)
            selp_np = bdarr.reshape(NB, 128, GB * B * BD)
        else:
            selp_pad = np.zeros((GPAD * 128, M, SP), dtype=BF16)
            selp_pad[:, :, :S] = selc.astype(BF16)
            # selp[k, nb*32+m, ((gq*B)+b)*SP + s]
            sc = selp_pad.reshape(NB, GB, B, 4, M, SP)
            selp_np = np.ascontiguousarray(
                sc.transpose(0, 3, 4, 1, 2, 5)).reshape(NB, 128, GB * B * SP)

        idxc = np.zeros((GPAD * 128, M), dtype=np.int64)
        idxc[:NPC] = idx_pad[core * NPC:(core + 1) * NPC]
        # idxp[k, nb*32+m, gq*B+b] = neighbor_idx[(4k+gq)*128 + b*4 + nb, m]
        idxp = np.ascontiguousarray(
            idxc.reshape(NB, GB, B, 4, M).transpose(0, 3, 4, 1, 2)
        ).reshape(NB, 128, GB * B)
        pfp_np = feat8u[idxp].reshape(NB, 128, GB * B * I)
        in_maps.append({
            "pfp": pfp_np,
            "selp": selp_np,
            "wt": wt_np,
            "zs": np.zeros((128, B * BD), dtype=BF16),
        })
    return in_maps


def run(feat_in, select_mat, weight, neighbor_idx, trace=False):
    nc = get_nc(SEL_FP8BD)
    in_maps = make_in_maps(feat_in, select_mat, weight, neighbor_idx, SEL_FP8BD)
    res = run_bass_kernel_spmd(nc, in_maps, core_ids=list(range(NCORES)), trace=trace)
    outs = [res.results[c]["outp"] for c in range(NCORES)]   # each [O, NPC]
    full = np.concatenate(outs, axis=1).astype(np.float32).T[:N]   # [N, O]
    return np.ascontiguousarray(full)[:, :, None], res


def kernel(feat_in, select_mat, weight, neighbor_idx):
    out, _ = run(feat_in, select_mat, weight, neighbor_idx, trace=False)
    return out


# revision 56
# speedup vs baseline: 1.0287x; 1.0105x over previous
"""CConv (continuous conv / GNN message passing) Trainium2 Bass kernel.

Math (per point n):
    pf[n,m,:]  = feat_in[neighbor_idx[n,m], :]                 # gather
    t[n,s,i]   = sum_m select_mat[n,m,s] * pf[n,m,i]           # stage 1
    out[n,o]   = sum_{s,i} t[n,s,i] * W[s,o,i]                 # stage 2

Strategy: data-parallel over points across 8 cores; per core, 49 groups of
128 points (32 blocks of 4 points). The neighbor gather runs host-side and
ships as an fp8e3 (e3m4) stream — half the HBM traffic of bf16; the fp8
scale folds into the stage-2 weight. pf and select arrive in 4-group DMA
chunks (~2 MB) so the SDMA engines run near line rate, issued only from
near-idle queues (sync/gpsimd) because an HWDGE issue parked in a busy
engine FIFO delays the prefetch by a whole batch. Two select paths:
SEL_FP8BD ships the select pre-block-diagonalized as fp8e3 (zero on-chip
prep); otherwise it ships dense bf16 and per-batch 4x-accelerated DVE
scatter copies build one of four persistent pre-zeroed block-diag
operands. Stage 1 is one matmul per 4-point block (lhsT = pf fp8, rhs =
112 block-diag cols) streaming at the PE's back-to-back rate into 2-bank
PSUM tiles (block stride padded to 128 cols so MM writes stay contiguous);
one 3-dim copy per tile drains both banks into a 4-group staging tile.
Stage 2 batches 4 groups: per spatial tap one N=512 matmul with the weight
stationary, accumulating in a full PSUM bank; the [O, points] output
layout is transposed back on the host.
"""
import sys

sys.path.insert(0, '/opt/trn_rl_repo')

import numpy as np
import ml_dtypes

import concourse.bass as bass
import concourse.tile as tile
from concourse import bacc, mybir
from concourse.bass_utils import run_bass_kernel_spmd

BF16 = ml_dtypes.bfloat16
F8E3 = ml_dtypes.float8_e3m4

N = 50000
M = 32            # neighbors per point
S = 27            # spatial bins
SP = 28           # padded spatial
I = 128           # in channels
O = 128           # out channels
NCORES = 8
NPAD = 50176      # 8 * 49 * 128
NPC = NPAD // NCORES        # 6272 points per core
G = NPC // 128              # 49 groups of 128 points
B = 32                      # 4-point blocks per group
SUB = 4                     # blocks per PSUM bank
BD = 4 * SP                 # block-diag columns per block (112)
GB = 4                      # groups per stage-2 batch / DMA chunk
NB = (G + GB - 1) // GB     # 13 batches (last has 1 group)
GPAD = NB * GB              # 52 groups incl. padding for whole DMA chunks

SEL_FP8BD = True            # ship select pre-block-diagonalized in fp8e3
SEL_BETA = 15.5             # select fp8 scale (only for SEL_FP8BD)


def build_nc(sel_fp8bd):
    nc = bacc.Bacc("TRN2", target_bir_lowering=False, debug=False)

    pfp = nc.dram_tensor("pfp", [NB, 128, GB * B * I], mybir.dt.uint8,
                         kind="ExternalInput")
    if sel_fp8bd:
        selp = nc.dram_tensor("selp", [NB, 128, GB * B * BD], mybir.dt.uint8,
                              kind="ExternalInput")
        SELW = B * BD
    else:
        selp = nc.dram_tensor("selp", [NB, 128, GB * B * SP], mybir.dt.bfloat16,
                              kind="ExternalInput")
        SELW = B * SP
    wt = nc.dram_tensor("wt", [I, S * O], mybir.dt.bfloat16, kind="ExternalInput")
    zs = nc.dram_tensor("zs", [128, B * BD], mybir.dt.bfloat16, kind="ExternalInput")
    outp = nc.dram_tensor("outp", [O, NPC], mybir.dt.bfloat16, kind="ExternalOutput")

    with tile.TileContext(nc) as tc:
        with (
            tc.tile_pool(name="const", bufs=1) as const_pool,
            tc.tile_pool(name="pfbuf", bufs=2) as pfbuf,
            tc.tile_pool(name="selbuf", bufs=2) as selbuf,
            tc.tile_pool(name="tgbuf", bufs=2) as tgbuf,
            tc.tile_pool(name="otbuf", bufs=2) as otbuf,
            tc.tile_pool(name="psum1", bufs=5, space="PSUM") as psum1,
            tc.tile_pool(name="psum2", bufs=3, space="PSUM") as psum2,
        ):
            wt_t = const_pool.tile([128, S * O], mybir.dt.bfloat16)
            nc.scalar.dma_start(out=wt_t[:], in_=wt[:])
            bd = None
            if not sel_fp8bd:
                # four persistent block-diag select operands (one per group
                # of a batch); zeros off the diagonal persist across groups
                bd0 = const_pool.tile([128, B, BD], mybir.dt.bfloat16)
                bd1 = const_pool.tile([128, B, BD], mybir.dt.bfloat16)
                bd2 = const_pool.tile([128, B, BD], mybir.dt.bfloat16)
                bd3 = const_pool.tile([128, B, BD], mybir.dt.bfloat16)
                bd = [bd0, bd1, bd2, bd3]
                nc.vector.memset(bd0[:], 0.0)
                nc.gpsimd.memset(bd1[:], 0.0)
                nc.scalar.copy(out=bd2[:], in_=bd0[:])
                nc.scalar.dma_start(out=bd3[:], in_=zs[:])

            def stage2(k, Tg4, ngrp):
                po = psum2.tile([128, GB * 128], mybir.dt.float32, space="PSUM")
                ncols = ngrp * 128
                for s in range(S):
                    nc.tensor.matmul(
                        out=po[:, 0:ncols],
                        lhsT=wt_t[:, s * O:(s + 1) * O],
                        rhs=Tg4[:, s, 0:ncols],
                        start=(s == 0), stop=(s == S - 1),
                    )
                ot = otbuf.tile([128, GB * 128], mybir.dt.bfloat16)
                if k % 2 == 0:
                    nc.vector.tensor_copy(out=ot[:, 0:ncols], in_=po[:, 0:ncols])
                else:
                    nc.scalar.copy(out=ot[:, 0:ncols], in_=po[:, 0:ncols])
                nc.sync.dma_start(out=outp[:, k * GB * 128:k * GB * 128 + ncols],
                                  in_=ot[:, 0:ncols])

            def alloc_tiles():
                pf_t = pfbuf.tile([128, GB, B, I], mybir.dt.uint8)
                if sel_fp8bd:
                    sel_t = selbuf.tile([128, GB, B, BD], mybir.dt.uint8)
                else:
                    sel_t = selbuf.tile([128, GB, B, SP], mybir.dt.bfloat16)
                return pf_t, sel_t

            # batch 0: per-group slices on the two HWDGE rings so group 0's
            # operands land in a few us (nothing else contends yet)
            cur = alloc_tiles()
            for gq in range(GB):
                nparts = 4 if gq == 0 else 1   # group 0: 8-block sub-slices
                for p in range(nparts):
                    bl, bh = p * B // nparts, (p + 1) * B // nparts
                    nc.scalar.dma_start(
                        out=cur[1][:, gq, bl:bh, :],
                        in_=selp[0][:, gq * SELW + bl * (SELW // B):
                                    gq * SELW + bh * (SELW // B)])
                    nc.sync.dma_start(
                        out=cur[0][:, gq, bl:bh, :],
                        in_=pfp[0][:, (gq * B + bl) * I:(gq * B + bh) * I])

            prev = None  # (batch index, Tg4 tile, ngroups)
            for k in range(NB):
                g0 = k * GB
                ngrp = min(GB, G - g0)
                pf_t, sel_t = cur

                Tg4 = tgbuf.tile([128, S, GB * 128], mybir.dt.bfloat16)
                if not sel_fp8bd:
                    # hoisted scatter burst: bd[gq][q, b, (q//32)*28 + s] =
                    # sel[q, gq, b, s]; keeps the DVE FIFO free of
                    # head-of-line waits on this batch's matmuls
                    for gq in range(ngrp):
                        for nb in range(4):
                            src = sel_t[32 * nb:32 * (nb + 1), gq, :, :]
                            dst = bd[gq][32 * nb:32 * (nb + 1), :,
                                         nb * SP:(nb + 1) * SP]
                            nc.vector.tensor_copy(out=dst, in_=src)
                for gq in range(ngrp):
                    g = g0 + gq
                    for c in range(B // SUB):
                        # 1-bank tile; block sub lives at col sub*128 (112
                        # data + 16 pad) -> contiguous MM writes AND a
                        # uniform-stride drain; 6 rotating banks + short
                        # per-bank drains keep the PE from stalling on
                        # PSUM recycling
                        pt = psum1.tile([128, 4, 128], mybir.dt.float32,
                                        space="PSUM")
                        for sub in range(SUB):
                            b = c * SUB + sub
                            if sel_fp8bd:
                                rhs_ap = sel_t[:, gq, b, :].bitcast(mybir.dt.float8e3)
                            else:
                                rhs_ap = bd[gq][:, b, :]
                            nc.tensor.matmul(
                                out=pt[:, sub, 0:BD],
                                lhsT=pf_t[:, gq, b, :].bitcast(mybir.dt.float8e3),
                                rhs=rhs_ap,
                                start=True, stop=True,
                            )
                        # drain: src col = sub*128 + nb*28 + s,
                        # dst col = s*512 + gq*128 + (c*4+sub)*4 + nb
                        src_ap = bass.AP(tensor=pt.tensor, offset=pt[:].offset,
                                         ap=[pt[:].ap[0], [1, S], [128, 4], [SP, 4]])
                        dst_ap = bass.AP(tensor=Tg4.tensor,
                                         offset=Tg4[:].offset + gq * 128 + c * 16,
                                         ap=[Tg4[:].ap[0], [GB * 128, S], [4, 4], [1, 4]])
                        if sel_fp8bd:
                            on_dve = c % 2 == 0
                        else:
                            # DVE runs the scatters too -> 3 of 8 drains
                            on_dve = c in (1, 4, 6)
                        if on_dve:
                            nc.vector.tensor_copy(out=dst_ap, in_=src_ap)
                        else:
                            nc.scalar.copy(out=dst_ap, in_=src_ap)

                    if gq == (0 if k == 0 else min(2, ngrp - 1)):
                        if k + 1 < NB:
                            # prefetch the next chunk now. k==0 issues from
                            # the ACT queue: its FIFO delays the transfer
                            # behind batch 0's first drains, so the big
                            # chunk doesn't steal SDMA bandwidth from batch
                            # 0's startup slices. Later batches use SWDGE
                            # (latency prefetch-hidden; pool WAR gates it).
                            nxt = alloc_tiles()
                            ring = nc.scalar if k == 0 else nc.gpsimd
                            ring.dma_start(out=nxt[0][:], in_=pfp[k + 1])
                            ring.dma_start(out=nxt[1][:], in_=selp[k + 1])
                            cur = nxt
                        if prev is not None:
                            stage2(*prev)
                prev = (k, Tg4, ngrp)
            stage2(*prev)

    nc.compile()
    return nc


_NC = {}


def get_nc(sel_fp8bd):
    if sel_fp8bd not in _NC:
        _NC[sel_fp8bd] = build_nc(sel_fp8bd)
    return _NC[sel_fp8bd]


def make_in_maps(feat_in, select_mat, weight, neighbor_idx, sel_fp8bd):
    feat = np.asarray(feat_in, dtype=np.float32)
    alpha = 15.5 / max(float(np.abs(feat).max()), 1e-30)
    feat8u = (feat * alpha).astype(F8E3).view(np.uint8)

    sel = np.asarray(select_mat, dtype=np.float32)
    sel_src = np.zeros((NPAD, M, S), dtype=np.float32)
    sel_src[:N] = sel

    nidx = np.asarray(neighbor_idx).astype(np.int64)
    idx_pad = np.zeros((NPAD, M), dtype=np.int64)
    idx_pad[:N] = nidx

    beta = SEL_BETA if sel_fp8bd else 1.0
    w = np.asarray(weight, dtype=np.float32)
    # wt[i, s*O+o] = W[s, o, i] / (alpha * beta)
    wt_np = np.ascontiguousarray(
        (w / (alpha * beta)).reshape(S, O, I).transpose(2, 0, 1).reshape(I, S * O)
    ).astype(BF16)

    in_maps = []
    for core in range(NCORES):
        selc = np.zeros((GPAD * 128, M, S), dtype=np.float32)
        selc[:NPC] = sel_src[core * NPC:(core + 1) * NPC]
        if sel_fp8bd:
            sel8 = (selc * beta).astype(F8E3).view(np.uint8)
            # selp[k, nb*32+m, ((gq*B)+b)*BD + nb*SP + s] =
            #     sel8[(4k+gq)*128 + b*4 + nb, m, s]
            sc = sel8.reshape(NB, GB, B, 4, M, S)
            bdarr = np.zeros((NB, 4, M, GB, B, 4, SP), dtype=np.uint8)
            for nb in range(4):
                bdarr[:, nb, :, :, :, nb, :S] = sc[:, :, :, nb, :, :].transpose(
                    0, 3, 1, 2, 4)
            selp_np = bdarr.reshape(NB, 128, GB * B * BD)
        else:
            selp_pad = np.zeros((GPAD * 128, M, SP), dtype=BF16)
            selp_pad[:, :, :S] = selc.astype(BF16)
            # selp[k, nb*32+m, ((gq*B)+b)*SP + s]
            sc = selp_pad.reshape(NB, GB, B, 4, M, SP)
            selp_np = np.ascontiguousarray(
                sc.transpose(0, 3, 4, 1, 2, 5)).reshape(NB, 128, GB * B * SP)

        idxc = np.zeros((GPAD * 128, M), dtype=np.int64)
        idxc[:NPC] = idx_pad[core * NPC:(core + 1) * NPC]
        # idxp[k, nb*32+m, gq*B+b] = neighbor_idx[(4k+gq)*128 + b*4 + nb, m]
        idxp = np.ascontiguousarray(
            idxc.reshape(NB, GB, B, 4, M).transpose(0, 3, 4, 1, 2)
        ).reshape(NB, 128, GB * B)
        pfp_np = feat8u[idxp].reshape(NB, 128, GB * B * I)
        in_maps.append({
            "pfp": pfp_np,
            "selp": selp_np,
            "wt": wt_np,
            "zs": np.zeros((128, B * BD), dtype=BF16),
        })
    return in_maps


def run(feat_in, select_mat, weight, neighbor_idx, trace=False):
    nc = get_nc(SEL_FP8BD)
    in_maps = make_in_maps(feat_in, select_mat, weight, neighbor_idx, SEL_FP8BD)
    res = run_bass_kernel_spmd(nc, in_maps, core_ids=list(range(NCORES)), trace=trace)
    outs = [res.results[c]["outp"] for c in range(NCORES)]   # each [O, NPC]
    full = np.concatenate(outs, axis=1).astype(np.float32).T[:N]   # [N, O]
    return np.ascontiguousarray(full)[:, :, None], res


def kernel(feat_in, select_mat, weight, neighbor_idx):
    out, _ = run(feat_in, select_mat, weight, neighbor_idx, trace=False)
    return out


# revision 57
# speedup vs baseline: 1.0728x; 1.0428x over previous
"""CConv (continuous conv / GNN message passing) Trainium2 Bass kernel.

Math (per point n):
    pf[n,m,:]  = feat_in[neighbor_idx[n,m], :]                 # gather
    t[n,s,i]   = sum_m select_mat[n,m,s] * pf[n,m,i]           # stage 1
    out[n,o]   = sum_{s,i} t[n,s,i] * W[s,o,i]                 # stage 2

Strategy: data-parallel over points across 8 cores; per core, 49 groups of
128 points (32 blocks of 4 points). The neighbor gather runs host-side and
ships as an fp8e3 (e3m4) stream — half the HBM traffic of bf16; the fp8
scale folds into the stage-2 weight. pf and select arrive in 4-group DMA
chunks (~2 MB) so the SDMA engines run near line rate, issued only from
near-idle queues (sync/gpsimd) because an HWDGE issue parked in a busy
engine FIFO delays the prefetch by a whole batch. Two select paths:
SEL_FP8BD ships the select pre-block-diagonalized as fp8e3 (zero on-chip
prep); otherwise it ships dense bf16 and per-batch 4x-accelerated DVE
scatter copies build one of four persistent pre-zeroed block-diag
operands. Stage 1 is one matmul per 4-point block (lhsT = pf fp8, rhs =
112 block-diag cols) streaming at the PE's back-to-back rate into 2-bank
PSUM tiles (block stride padded to 128 cols so MM writes stay contiguous);
one 3-dim copy per tile drains both banks into a 4-group staging tile.
Stage 2 batches 4 groups: per spatial tap one N=512 matmul with the weight
stationary, accumulating in a full PSUM bank; the [O, points] output
layout is transposed back on the host.
"""
import sys

sys.path.insert(0, '/opt/trn_rl_repo')

import numpy as np
import ml_dtypes

import concourse.bass as bass
import concourse.tile as tile
from concourse import bacc, mybir
from concourse.bass_utils import run_bass_kernel_spmd

BF16 = ml_dtypes.bfloat16
F8E3 = ml_dtypes.float8_e3m4

N = 50000
M = 32            # neighbors per point
S = 27            # spatial bins
SP = 28           # padded spatial
I = 128           # in channels
O = 128           # out channels
NCORES = 8
NPAD = 50176      # 8 * 49 * 128
NPC = NPAD // NCORES        # 6272 points per core
G = NPC // 128              # 49 groups of 128 points
B = 32                      # 4-point blocks per group
SUB = 4                     # blocks per PSUM bank
BD = 4 * SP                 # block-diag columns per block (112)
GB = 4                      # groups per stage-2 batch / DMA chunk
NB = (G + GB - 1) // GB     # 13 batches (last has 1 group)
GPAD = NB * GB              # 52 groups incl. padding for whole DMA chunks

SEL_FP8BD = True            # ship select pre-block-diagonalized in fp8e3
SEL_BETA = 15.5             # select fp8 scale (only for SEL_FP8BD)


def build_nc(sel_fp8bd):
    nc = bacc.Bacc("TRN2", target_bir_lowering=False, debug=False)

    pfp = nc.dram_tensor("pfp", [NB, 128, GB * B * I], mybir.dt.uint8,
                         kind="ExternalInput")
    if sel_fp8bd:
        selp = nc.dram_tensor("selp", [NB, 128, GB * B * BD], mybir.dt.uint8,
                              kind="ExternalInput")
        SELW = B * BD
    else:
        selp = nc.dram_tensor("selp", [NB, 128, GB * B * SP], mybir.dt.bfloat16,
                              kind="ExternalInput")
        SELW = B * SP
    wt = nc.dram_tensor("wt", [I, S * O], mybir.dt.bfloat16, kind="ExternalInput")
    zs = nc.dram_tensor("zs", [128, B * BD], mybir.dt.bfloat16, kind="ExternalInput")
    outp = nc.dram_tensor("outp", [O, NPC], mybir.dt.bfloat16, kind="ExternalOutput")

    with tile.TileContext(nc) as tc:
        with (
            tc.tile_pool(name="const", bufs=1) as const_pool,
            tc.tile_pool(name="pfbuf", bufs=2) as pfbuf,
            tc.tile_pool(name="selbuf", bufs=2) as selbuf,
            tc.tile_pool(name="tgbuf", bufs=2) as tgbuf,
            tc.tile_pool(name="otbuf", bufs=2) as otbuf,
            tc.tile_pool(name="psum1", bufs=5, space="PSUM") as psum1,
            tc.tile_pool(name="psum2", bufs=3, space="PSUM") as psum2,
        ):
            wt_t = const_pool.tile([128, S * O], mybir.dt.bfloat16)
            nc.scalar.dma_start(out=wt_t[:], in_=wt[:])
            bd = None
            if not sel_fp8bd:
                # four persistent block-diag select operands (one per group
                # of a batch); zeros off the diagonal persist across groups
                bd0 = const_pool.tile([128, B, BD], mybir.dt.bfloat16)
                bd1 = const_pool.tile([128, B, BD], mybir.dt.bfloat16)
                bd2 = const_pool.tile([128, B, BD], mybir.dt.bfloat16)
                bd3 = const_pool.tile([128, B, BD], mybir.dt.bfloat16)
                bd = [bd0, bd1, bd2, bd3]
                nc.vector.memset(bd0[:], 0.0)
                nc.gpsimd.memset(bd1[:], 0.0)
                nc.scalar.copy(out=bd2[:], in_=bd0[:])
                nc.scalar.dma_start(out=bd3[:], in_=zs[:])

            def stage2(k, Tg4, ngrp):
                po = psum2.tile([128, GB * 128], mybir.dt.float32, space="PSUM")
                ncols = ngrp * 128
                for s in range(S):
                    nc.tensor.matmul(
                        out=po[:, 0:ncols],
                        lhsT=wt_t[:, s * O:(s + 1) * O],
                        rhs=Tg4[:, s, 0:ncols],
                        start=(s == 0), stop=(s == S - 1),
                    )
                ot = otbuf.tile([128, GB * 128], mybir.dt.bfloat16)
                if k % 2 == 0:
                    nc.vector.tensor_copy(out=ot[:, 0:ncols], in_=po[:, 0:ncols])
                else:
                    nc.scalar.copy(out=ot[:, 0:ncols], in_=po[:, 0:ncols])
                nc.sync.dma_start(out=outp[:, k * GB * 128:k * GB * 128 + ncols],
                                  in_=ot[:, 0:ncols])

            def alloc_tiles():
                pf_t = pfbuf.tile([128, GB, B, I], mybir.dt.uint8)
                if sel_fp8bd:
                    sel_t = selbuf.tile([128, GB, B, BD], mybir.dt.uint8)
                else:
                    sel_t = selbuf.tile([128, GB, B, SP], mybir.dt.bfloat16)
                return pf_t, sel_t

            # batch 0: per-group slices on the two HWDGE rings so group 0's
            # operands land in a few us (nothing else contends yet)
            cur = alloc_tiles()
            for gq in range(GB):
                nparts = 4 if gq == 0 else 1   # group 0: 8-block sub-slices
                for p in range(nparts):
                    bl, bh = p * B // nparts, (p + 1) * B // nparts
                    nc.scalar.dma_start(
                        out=cur[1][:, gq, bl:bh, :],
                        in_=selp[0][:, gq * SELW + bl * (SELW // B):
                                    gq * SELW + bh * (SELW // B)])
                    nc.sync.dma_start(
                        out=cur[0][:, gq, bl:bh, :],
                        in_=pfp[0][:, (gq * B + bl) * I:(gq * B + bh) * I])

            prev = None  # (batch index, Tg4 tile, ngroups)
            for k in range(NB):
                g0 = k * GB
                ngrp = min(GB, G - g0)
                pf_t, sel_t = cur

                Tg4 = tgbuf.tile([128, S, GB * 128], mybir.dt.bfloat16)
                if not sel_fp8bd:
                    # hoisted scatter burst: bd[gq][q, b, (q//32)*28 + s] =
                    # sel[q, gq, b, s]; keeps the DVE FIFO free of
                    # head-of-line waits on this batch's matmuls
                    for gq in range(ngrp):
                        for nb in range(4):
                            src = sel_t[32 * nb:32 * (nb + 1), gq, :, :]
                            dst = bd[gq][32 * nb:32 * (nb + 1), :,
                                         nb * SP:(nb + 1) * SP]
                            nc.vector.tensor_copy(out=dst, in_=src)
                for gq in range(ngrp):
                    g = g0 + gq
                    for c in range(B // SUB):
                        # 1-bank tile; block sub lives at col sub*128 (112
                        # data + 16 pad) -> contiguous MM writes AND a
                        # uniform-stride drain; 6 rotating banks + short
                        # per-bank drains keep the PE from stalling on
                        # PSUM recycling
                        pt = psum1.tile([128, 4, 128], mybir.dt.float32,
                                        space="PSUM")
                        for sub in range(SUB):
                            b = c * SUB + sub
                            if sel_fp8bd:
                                rhs_ap = sel_t[:, gq, b, :].bitcast(mybir.dt.float8e3)
                            else:
                                rhs_ap = bd[gq][:, b, :]
                            nc.tensor.matmul(
                                out=pt[:, sub, 0:BD],
                                lhsT=pf_t[:, gq, b, :].bitcast(mybir.dt.float8e3),
                                rhs=rhs_ap,
                                start=True, stop=True,
                            )
                        # drain: src col = sub*128 + nb*28 + s,
                        # dst col = s*512 + gq*128 + (c*4+sub)*4 + nb
                        src_ap = bass.AP(tensor=pt.tensor, offset=pt[:].offset,
                                         ap=[pt[:].ap[0], [1, S], [128, 4], [SP, 4]])
                        dst_ap = bass.AP(tensor=Tg4.tensor,
                                         offset=Tg4[:].offset + gq * 128 + c * 16,
                                         ap=[Tg4[:].ap[0], [GB * 128, S], [4, 4], [1, 4]])
                        if sel_fp8bd:
                            on_dve = c % 2 == 0
                        else:
                            # DVE runs the scatters too -> 3 of 8 drains
                            on_dve = c in (1, 4, 6)
                        if on_dve:
                            nc.vector.tensor_copy(out=dst_ap, in_=src_ap)
                        else:
                            nc.scalar.copy(out=dst_ap, in_=src_ap)

                    if gq == (0 if k == 0 else min(1, ngrp - 1)):
                        if k + 1 < NB:
                            # prefetch the next chunk now. k==0 issues from
                            # the ACT queue: its FIFO delays the transfer
                            # behind batch 0's first drains, so the big
                            # chunk doesn't steal SDMA bandwidth from batch
                            # 0's startup slices. Later batches use SWDGE
                            # (latency prefetch-hidden; pool WAR gates it).
                            nxt = alloc_tiles()
                            ring = nc.scalar if k == 0 else nc.gpsimd
                            ring.dma_start(out=nxt[0][:], in_=pfp[k + 1])
                            ring.dma_start(out=nxt[1][:], in_=selp[k + 1])
                            cur = nxt
                        if prev is not None:
                            stage2(*prev)
                prev = (k, Tg4, ngrp)
            stage2(*prev)

    nc.compile()
    return nc


_NC = {}


def get_nc(sel_fp8bd):
    if sel_fp8bd not in _NC:
        _NC[sel_fp8bd] = build_nc(sel_fp8bd)
    return _NC[sel_fp8bd]


def make_in_maps(feat_in, select_mat, weight, neighbor_idx, sel_fp8bd):
    feat = np.asarray(feat_in, dtype=np.float32)
    alpha = 15.5 / max(float(np.abs(feat).max()), 1e-30)
    feat8u = (feat * alpha).astype(F8E3).view(np.uint8)

    sel = np.asarray(select_mat, dtype=np.float32)
    sel_src = np.zeros((NPAD, M, S), dtype=np.float32)
    sel_src[:N] = sel

    nidx = np.asarray(neighbor_idx).astype(np.int64)
    idx_pad = np.zeros((NPAD, M), dtype=np.int64)
    idx_pad[:N] = nidx

    beta = SEL_BETA if sel_fp8bd else 1.0
    w = np.asarray(weight, dtype=np.float32)
    # wt[i, s*O+o] = W[s, o, i] / (alpha * beta)
    wt_np = np.ascontiguousarray(
        (w / (alpha * beta)).reshape(S, O, I).transpose(2, 0, 1).reshape(I, S * O)
    ).astype(BF16)

    in_maps = []
    for core in range(NCORES):
        selc = np.zeros((GPAD * 128, M, S), dtype=np.float32)
        selc[:NPC] = sel_src[core * NPC:(core + 1) * NPC]
        if sel_fp8bd:
            sel8 = (selc * beta).astype(F8E3).view(np.uint8)
            # selp[k, nb*32+m, ((gq*B)+b)*BD + nb*SP + s] =
            #     sel8[(4k+gq)*128 + b*4 + nb, m, s]
            sc = sel8.reshape(NB, GB, B, 4, M, S)
            bdarr = np.zeros((NB, 4, M, GB, B, 4, SP), dtype=np.uint8)
            for nb in range(4):
                bdarr[:, nb, :, :, :, nb, :S] = sc[:, :, :, nb, :, :].transpose(
                    0, 3, 1, 2, 4)
            selp_np = bdarr.reshape(NB, 128, GB * B * BD)
        else:
            selp_pad = np.zeros((GPAD * 128, M, SP), dtype=BF16)
            selp_pad[:, :, :S] = selc.astype(BF16)
            # selp[k, nb*32+m, ((gq*B)+b)*SP + s]
            sc = selp_pad.reshape(NB, GB, B, 4, M, SP)
            selp_np = np.ascontiguousarray(
                sc.transpose(0, 3, 4, 1, 2, 5)).reshape(NB, 128, GB * B * SP)

        idxc = np.zeros((GPAD * 128, M), dtype=np.int64)
        idxc[:NPC] = idx_pad[core * NPC:(core + 1) * NPC]
        # idxp[k, nb*32+m, gq*B+b] = neighbor_idx[(4k+gq)*128 + b*4 + nb, m]
        idxp = np.ascontiguousarray(
            idxc.reshape(NB, GB, B, 4, M).transpose(0, 3, 4, 1, 2)
        ).reshape(NB, 128, GB * B)
        pfp_np = feat8u[idxp].reshape(NB, 128, GB * B * I)
        in_maps.append({
            "pfp": pfp_np,
            "selp": selp_np,
            "wt": wt_np,
            "zs": np.zeros((128, B * BD), dtype=BF16),
        })
    return in_maps


def run(feat_in, select_mat, weight, neighbor_idx, trace=False):
    nc = get_nc(SEL_FP8BD)
    in_maps = make_in_maps(feat_in, select_mat, weight, neighbor_idx, SEL_FP8BD)
    res = run_bass_kernel_spmd(nc, in_maps, core_ids=list(range(NCORES)), trace=trace)
    outs = [res.results[c]["outp"] for c in range(NCORES)]   # each [O, NPC]
    full = np.concatenate(outs, axis=1).astype(np.float32).T[:N]   # [N, O]
    return np.ascontiguousarray(full)[:, :, None], res


def kernel(feat_in, select_mat, weight, neighbor_idx):
    out, _ = run(feat_in, select_mat, weight, neighbor_idx, trace=False)
    return out
